# revision 13
# baseline (speedup 1.0000x reference)
"""Trainium2 Bass kernel for the MessageBlock GNN message-passing layer.

Strategy (8 NeuronCores, no collectives):
  - Sort edges by destination node on host; shard by dst range: core c owns
    nodes [c*1250, (c+1)*1250), split into 10 blocks of 128 nodes.
  - Node MLP phi = Linear(SiLU(Linear(s))) computed per *node* (not per edge)
    on every core (redundant but cheap), written to a DRAM table in bf16.
  - Per edge tile (128 edges): dma_gather phi[src], vec[src] (bf16 rows),
    rbf @ WrbfT via row-packed matmuls, fcut fold via ScalarE, elementwise
    message assembly on VectorE/GpSimd, then scatter-add via one-hot S
    matmuls accumulating in PSUM per 128-node block.
  - Each core writes its own disjoint slice of ds/dvec; host concatenates.
"""

import sys
import os

for _p in ("/opt/trn_rl_repo", "/root/.axon_site/_ro/trn_rl_repo"):
    if os.path.isdir(_p) and _p not in sys.path:
        sys.path.insert(0, _p)

import numpy as np
import ml_dtypes

BF16 = ml_dtypes.bfloat16

N_NODES = 10000
F = 128
RBF = 20
N_CORES = 8
NPC = 1250            # nodes per core
NBLK = 10             # node blocks per core (128 nodes each; last=98)
BLKW = 128            # block width (nodes)
G = 4                 # tiles per elementwise supergroup
GCH = 8               # tiles per gather chunk
NT_PAD = 10240        # padded node count for the MLP phase (80 tiles of 128)


def _split_bf16(x):
    hi = x.astype(BF16)
    lo = (x.astype(np.float32) - hi.astype(np.float32)).astype(BF16)
    return hi, lo


def _wrap_idx(idx_i16, ncols):
    """dma_gather index layout: idx i -> partition i%16, col i//16,
    replicated across the 8 Q7 cores (partitions 16..127)."""
    out = np.zeros((128, ncols), np.int16)
    n = len(idx_i16)
    cols = (n + 15) // 16
    tmp = np.full(16 * cols, 0, np.int16)
    tmp[:n] = idx_i16
    out[:16, :cols] = tmp.reshape(cols, 16).T
    return np.tile(out[:16], (8, 1))



def _install_ntff_hook_shim():
    """The image's antenv package lacks axon_hooks; synthesize it and
    register the boot-provided ctypes NTFF profiling hook."""
    import types
    import antenv
    if "antenv.axon_hooks" in sys.modules:
        return
    mod = types.ModuleType("antenv.axon_hooks")
    mod._hook = None

    def set_axon_ntff_profile_hook(h):
        mod._hook = h

    def get_axon_ntff_profile_hook():
        return mod._hook

    mod.set_axon_ntff_profile_hook = set_axon_ntff_profile_hook
    mod.get_axon_ntff_profile_hook = get_axon_ntff_profile_hook
    sys.modules["antenv.axon_hooks"] = mod
    antenv.axon_hooks = mod
    try:
        from trn_agent_boot.trn_boot import _ntff_profile_via_ctypes
        hook = _ntff_profile_via_ctypes("/opt/axon/libaxon_pjrt.so")
        if hook is not None:
            mod._hook = hook
    except Exception as e:
        print(f"ntff hook shim failed: {e}")


_PROGRAM_CACHE = {}


def _build_program(T_bs, brbf_nz, bs2_nz):
    key = (tuple(T_bs), brbf_nz, bs2_nz, os.environ.get("GNN_PREP", "1"))
    if key in _PROGRAM_CACHE:
        return _PROGRAM_CACHE[key]

    import concourse.bass as bass
    import concourse.bacc as bacc
    import concourse.mybir as mybir
    import concourse.tile as tile
    from concourse.tile import add_dep_helper

    dt = mybir.dt
    ALU = mybir.AluOpType
    ACTF = mybir.ActivationFunctionType

    T_total = sum(T_bs)
    KRBF = 21 if brbf_nz else 20

    nc = bacc.Bacc("TRN2", target_bir_lowering=False, debug=False, num_swdge_queues=4)

    # ---- external inputs (identical shapes on every core) ----
    sT_hi_d = nc.dram_tensor("sT_hi", [128, NT_PAD], dt.bfloat16, kind="ExternalInput")
    sT_lo_d = nc.dram_tensor("sT_lo", [128, NT_PAD], dt.bfloat16, kind="ExternalInput")
    w1T_hi_d = nc.dram_tensor("w1T_hi", [128, 128], dt.bfloat16, kind="ExternalInput")
    w1T_lo_d = nc.dram_tensor("w1T_lo", [128, 128], dt.bfloat16, kind="ExternalInput")
    w2T_hi_d = nc.dram_tensor("w2T_hi", [128, 384], dt.bfloat16, kind="ExternalInput")
    w2T_lo_d = nc.dram_tensor("w2T_lo", [128, 384], dt.bfloat16, kind="ExternalInput")
    bs1_d = nc.dram_tensor("bs1c", [128, 1], dt.float32, kind="ExternalInput")
    if bs2_nz:
        bs2hl_d = nc.dram_tensor("bs2hl", [2, 384], dt.bfloat16, kind="ExternalInput")
        ones2_d = nc.dram_tensor("ones2", [2, 128], dt.bfloat16, kind="ExternalInput")
    vec_tab_d = nc.dram_tensor("vec_tab", [N_NODES, 384], dt.bfloat16, kind="ExternalInput")
    wrbf_d = nc.dram_tensor("wrbf_rep", [128, 384], dt.bfloat16, kind="ExternalInput")
    rbf2T_d = nc.dram_tensor("rbf2T", [64, (T_total // 2) * 128], dt.bfloat16, kind="ExternalInput")
    idx_d = nc.dram_tensor("idx_all", [128, T_total * 8], dt.int16, kind="ExternalInput")
    S_d = nc.dram_tensor("S_bf", [128, T_total * 128], dt.bfloat16, kind="ExternalInput")
    dist_d = nc.dram_tensor("dist_pt", [128, T_total], dt.float32, kind="ExternalInput")
    ev_d = nc.dram_tensor("ev_pt", [128, T_total * 3], dt.float32, kind="ExternalInput")
    rcinfo_d = nc.dram_tensor("rcinfo", [128, 4], dt.float32, kind="ExternalInput")

    ds_out_d = nc.dram_tensor("ds_out", [NBLK * 128, 128], dt.float32, kind="ExternalOutput")
    dv_out_d = nc.dram_tensor("dv_out", [NBLK * 128, 384], dt.float32, kind="ExternalOutput")

    # internal DRAM: combined gather table rows = [phi(384) | vec(384)] bf16
    tab_d = nc.dram_tensor("tab", [NT_PAD, 768], dt.bfloat16)

    NCHUNK = NT_PAD // 512    # 20 node chunks for mm1
    NTILES = NT_PAD // 128    # 80 node tiles for mm2

    with tile.TileContext(nc) as tc:
        # ---------------- constants / persistent tiles ----------------
        with tc.tile_pool(name="const", bufs=1) as cp:
            w1T_hi = cp.tile([128, 128], dt.bfloat16, tag="w1hi")
            w1T_lo = cp.tile([128, 128], dt.bfloat16, tag="w1lo")
            w2T_hi = cp.tile([128, 384], dt.bfloat16, tag="w2hi")
            w2T_lo = cp.tile([128, 384], dt.bfloat16, tag="w2lo")
            bs1 = cp.tile([128, 1], dt.float32, tag="bs1")
            wrbf = cp.tile([128, 384], dt.bfloat16, tag="wrbf")
            rbf2T = cp.tile([64, (T_total // 2) * 128], dt.bfloat16, tag="rbf2T")
            idx_all = cp.tile([128, T_total * 8], dt.int16, tag="idx")
            dist = cp.tile([128, T_total], dt.float32, tag="dist")
            ev = cp.tile([128, T_total * 3], dt.float32, tag="ev")
            rcinfo = cp.tile([128, 4], dt.float32, tag="rcinfo")
            fcut = cp.tile([128, T_total], dt.float32, tag="fcut")
            vecn = cp.tile([128, T_total * 3], dt.float32, tag="vecn")
            scr0 = cp.tile([128, T_total], dt.float32, tag="scr0")
            scr1 = cp.tile([128, T_total], dt.float32, tag="scr1")
            scr2 = cp.tile([128, T_total], dt.float32, tag="scr2")
            if bs2_nz:
                bs2hl = cp.tile([2, 384], dt.bfloat16, tag="bs2hl")
                ones2 = cp.tile([2, 128], dt.bfloat16, tag="ones2")

            nc.sync.dma_start(out=w1T_hi[:], in_=w1T_hi_d[:])
            nc.sync.dma_start(out=w1T_lo[:], in_=w1T_lo_d[:])
            nc.sync.dma_start(out=w2T_hi[:], in_=w2T_hi_d[:])
            nc.sync.dma_start(out=w2T_lo[:], in_=w2T_lo_d[:])
            nc.sync.dma_start(out=bs1[:], in_=bs1_d[:])
            nc.sync.dma_start(out=wrbf[:], in_=wrbf_d[:])
            nc.sync.dma_start(out=rbf2T[:], in_=rbf2T_d[:])
            nc.scalar.dma_start(out=idx_all[:], in_=idx_d[:])
            nc.scalar.dma_start(out=dist[:], in_=dist_d[:])
            nc.scalar.dma_start(out=ev[:], in_=ev_d[:])
            nc.sync.dma_start(out=rcinfo[:], in_=rcinfo_d[:])
            # vec half of the gather table (phi half written by node phase)
            nc.gpsimd.dma_start(out=tab_d[0:N_NODES, 384:768], in_=vec_tab_d[:])
            if bs2_nz:
                nc.sync.dma_start(out=bs2hl[:], in_=bs2hl_d[:])
                nc.sync.dma_start(out=ones2[:], in_=ones2_d[:])

            # ---------------- edge-scalar precompute ----------------
            # fcut = 0.5*(cos(pi*d/rc)+1)*(d<rc) = (1 - sin^2(pi*d/(2rc)))*(d<rc)
            # sin((pi/2)u) = u * Q(u^2), Taylor with 7 terms (|err| ~ 1e-9 on [0,1]).
            # rcinfo col0 = 1/rc, col1 = rc.  ACT Sin LUT is inaccurate; use DVE poly.
            import math
            acoef = [(-1.0) ** kk * (math.pi / 2) ** (2 * kk + 1) / math.factorial(2 * kk + 1)
                     for kk in range(7)]
            u_t = scr0
            u2_t = scr1
            q_t = scr2
            nc.vector.tensor_scalar(
                out=u_t[:], in0=dist[:], scalar1=rcinfo[:, 0:1], scalar2=None,
                op0=ALU.mult)
            nc.vector.tensor_tensor(out=u2_t[:], in0=u_t[:], in1=u_t[:], op=ALU.mult)
            nc.vector.tensor_scalar(
                out=q_t[:], in0=u2_t[:], scalar1=float(acoef[6]), scalar2=float(acoef[5]),
                op0=ALU.mult, op1=ALU.add)
            for kk in (4, 3, 2, 1, 0):
                nc.vector.tensor_tensor(out=q_t[:], in0=q_t[:], in1=u2_t[:], op=ALU.mult)
                nc.vector.tensor_scalar(
                    out=q_t[:], in0=q_t[:], scalar1=float(acoef[kk]), scalar2=None,
                    op0=ALU.add)
            nc.vector.tensor_tensor(out=q_t[:], in0=q_t[:], in1=u_t[:], op=ALU.mult)
            nc.vector.tensor_tensor(out=q_t[:], in0=q_t[:], in1=q_t[:], op=ALU.mult)
            nc.vector.tensor_scalar(
                out=q_t[:], in0=q_t[:], scalar1=-1.0, scalar2=1.0,
                op0=ALU.mult, op1=ALU.add)
            nc.vector.tensor_scalar(
                out=scr0[:], in0=dist[:], scalar1=rcinfo[:, 1:2], scalar2=None,
                op0=ALU.is_lt)
            nc.vector.tensor_tensor(out=fcut[:], in0=q_t[:], in1=scr0[:], op=ALU.mult)
            # vecn[:, 3t+j] = ev[:, 3t+j] / dist[:, t]
            nc.vector.reciprocal(out=scr1[:], in_=dist[:])
            ev3 = ev[:].rearrange("p (t j) -> p t j", j=3)
            vecn3 = vecn[:].rearrange("p (t j) -> p t j", j=3)
            for j in range(3):
                nc.vector.tensor_tensor(
                    out=vecn3[:, :, j], in0=ev3[:, :, j], in1=scr1[:], op=ALU.mult
                )

            # ---------------- phase 1: node MLP ----------------
            with tc.tile_pool(name="nodephase", bufs=1) as npp, \
                 tc.tile_pool(name="phistage", bufs=4) as php, \
                 tc.tile_pool(name="ps_node", bufs=2, space="PSUM") as psn:
                sT_hi = npp.tile([128, NT_PAD], dt.bfloat16, tag="sThi")
                sT_lo = npp.tile([128, NT_PAD], dt.bfloat16, tag="sTlo")
                h_bf = npp.tile([128, NT_PAD], dt.bfloat16, tag="hbf")
                nc.sync.dma_start(out=sT_hi[:], in_=sT_hi_d[:])
                nc.sync.dma_start(out=sT_lo[:], in_=sT_lo_d[:])

                for ch in range(NCHUNK):
                    sl = slice(ch * 512, (ch + 1) * 512)
                    h_ps = psn.tile([128, 512], dt.float32, tag="h")
                    nc.tensor.matmul(out=h_ps[:], lhsT=w1T_hi[:], rhs=sT_hi[:, sl],
                                     start=True, stop=False)
                    nc.tensor.matmul(out=h_ps[:], lhsT=w1T_hi[:], rhs=sT_lo[:, sl],
                                     start=False, stop=False)
                    nc.tensor.matmul(out=h_ps[:], lhsT=w1T_lo[:], rhs=sT_hi[:, sl],
                                     start=False, stop=True)
                    nc.scalar.activation(out=h_bf[:, sl], in_=h_ps[:],
                                         func=ACTF.Silu, bias=bs1[:, 0:1], scale=1.0)

                for ntile in range(NTILES):
                    nsl = slice(ntile * 128, (ntile + 1) * 128)
                    phi_ps = psn.tile([128, 384], dt.float32, tag="phi")
                    nc.tensor.matmul(out=phi_ps[:], lhsT=h_bf[:, nsl], rhs=w2T_hi[:],
                                     start=True, stop=False)
                    nc.tensor.matmul(out=phi_ps[:], lhsT=h_bf[:, nsl], rhs=w2T_lo[:],
                                     start=False, stop=not bs2_nz)
                    if bs2_nz:
                        nc.tensor.matmul(out=phi_ps[:], lhsT=ones2[:], rhs=bs2hl[:],
                                         start=False, stop=True)
                    phi_sb = php.tile([128, 384], dt.bfloat16, tag="phisb")
                    if ntile % 2 == 0:
                        nc.scalar.activation(out=phi_sb[:], in_=phi_ps[:], func=ACTF.Copy)
                    else:
                        nc.vector.tensor_copy(out=phi_sb[:], in_=phi_ps[:])
                    nc.sync.dma_start(out=tab_d[nsl, 0:384], in_=phi_sb[:])

            # fence: a tiny read of tab_d orders gather triggers after all
            # table writers (phi-phase DMAs + vec copy); the PREPARE_ONLY
    # trigger path does not carry the DRAM RAW itself.
            fence_t = cp.tile([128, 8], dt.bfloat16, tag="fence")
            fence_ins = nc.sync.dma_start(out=fence_t[:], in_=tab_d[0:128, 0:8]).ins

            # ---------------- phase 2: edge processing ----------------
            with tc.tile_pool(name="gath", bufs=6) as gp, \
                 tc.tile_pool(name="sblk", bufs=2) as sp, \
                 tc.tile_pool(name="msgp", bufs=2) as mp, \
                 tc.tile_pool(name="wbfp", bufs=2) as wp, \
                 tc.tile_pool(name="outp", bufs=2) as op_, \
                 tc.tile_pool(name="ps_w", bufs=2, space="PSUM") as psw, \
                 tc.tile_pool(name="ps_acc", bufs=2, space="PSUM") as psa:

                t_base = 0
                gq = 0
                GBUFS = 6
                use_prep = bool(int(os.environ.get("GNN_PREP", "1")))
                slot_readers = [[] for _ in range(GBUFS)]
                for b in range(NBLK):
                    T_b = T_bs[b]
                    nchunks = T_b // GCH
                    chunk_tiles = []
                    chunk_waits = []
                    chunk_slots = []
                    for chi in range(nchunks):
                        ct0 = chi * GCH
                        nidx = GCH * 128
                        isl = slice((t_base + ct0) * 8, (t_base + ct0 + GCH) * 8)
                        gt = gp.tile([128, GCH, 768], dt.bfloat16, tag="gath")
                        slot = gq % GBUFS
                        if use_prep:
                            gsem = nc.alloc_semaphore(f"gsem_{b}_{chi}")
                            nc.gpsimd.dma_gather(
                                gt[:], tab_d[:], idx_all[:, isl],
                                nidx, nidx, 768, single_packet=False,
                                queue_num=gq % 4, prepare_only=True, sem=gsem)
                            trig = nc.gpsimd.trigger_dma(count=1, queue_num=gq % 4)
                            add_dep_helper(trig.ins, fence_ins, sync=True,
                                           reason="gather after table writes")
                            for r in slot_readers[slot]:
                                add_dep_helper(trig.ins, r, sync=True,
                                               reason="WAR prior chunk readers")
                            w = nc.vector.wait_ge(gsem, 16)
                            chunk_waits.append(w.ins)
                        else:
                            nc.gpsimd.dma_gather(
                                gt[:], tab_d[:], idx_all[:, isl],
                                nidx, nidx, 768, single_packet=False,
                                queue_num=gq % 4)
                            chunk_waits.append(None)
                        slot_readers[slot] = []
                        chunk_slots.append(slot)
                        gq += 1
                        chunk_tiles.append(gt)

                    S_blk = sp.tile([128, T_b * 128], dt.bfloat16, tag="S")
                    nc.scalar.dma_start(
                        out=S_blk[:], in_=S_d[:, t_base * 128:(t_base + T_b) * 128])

                    accA = psa.tile([128, 512], dt.float32, tag="accA")
                    accB = psa.tile([128, 384], dt.float32, tag="accB")

                    for g in range(T_b // G):
                        gt0 = g * G            # tile index within block
                        MSG = mp.tile([128, G, 1152], dt.bfloat16, tag="MSG")
                        W_bf = wp.tile([128, G, 384], dt.bfloat16, tag="Wbf")
                        wps = psw.tile([128, 2, 512], dt.float32, tag="wps")
                        # rbf matmuls: pack 2 tiles into PE rows (0,32)
                        for i in range(G):
                            t_glob = t_base + gt0 + i
                            grp = t_glob % 2
                            q = t_glob // 2
                            nc.tensor.matmul(
                                out=wps[:, i % 2, 0:384],
                                lhsT=rbf2T[32 * grp:32 * grp + KRBF,
                                           q * 128:(q + 1) * 128],
                                rhs=wrbf[32 * grp:32 * grp + KRBF, :],
                                start=True, stop=True)
                            if i % 2 == 1 or i == G - 1:
                                for ii in (i - i % 2, i) if i % 2 == 1 else (i,):
                                    tt = t_base + gt0 + ii
                                    nc.scalar.activation(
                                        out=W_bf[:, ii, :], in_=wps[:, ii % 2, 0:384],
                                        func=ACTF.Copy, scale=fcut[:, tt:tt + 1])
                                if i != G - 1:
                                    wps = psw.tile([128, 2, 512], dt.float32, tag="wps")
                        chi = gt0 // GCH
                        gt = chunk_tiles[chi]
                        goff = gt0 % GCH
                        readers = slot_readers[chunk_slots[chi]]
                        # phiW = phi * W  -> MSG[:, :, 0:384]  ([Wvv|Wvs|ds])
                        phiw_tt = nc.vector.tensor_tensor(
                            out=MSG[:, :, 0:384],
                            in0=gt[:, goff:goff + G, 0:384],
                            in1=W_bf[:], op=ALU.mult)
                        if chunk_waits[chi] is not None:
                            add_dep_helper(phiw_tt.ins, chunk_waits[chi], sync=False,
                                           reason="gather data wait")
                        readers.append(phiw_tt.ins)
                        # t1_j = vec_j * Wvv -> MSG[:, :, 384+128j : ...]
                        for j in range(3):
                            eng = nc.vector if j == 0 else nc.gpsimd
                            t1_tt = eng.tensor_tensor(
                                out=MSG[:, :, 384 + 128 * j:384 + 128 * (j + 1)],
                                in0=gt[:, goff:goff + G,
                                       384 + 128 * j:384 + 128 * (j + 1)],
                                in1=MSG[:, :, 0:128], op=ALU.mult)
                            readers.append(t1_tt.ins)
                        # t2 = vecn_bcast * Wvs_bcast in one TT over [128, G, 3, 128]
                        t_glob0 = t_base + gt0
                        vslice = vecn[:, 3 * t_glob0:3 * (t_glob0 + G)]
                        nc.vector.tensor_tensor(
                            out=MSG[:, :, 768:1152].rearrange("p g (j f) -> p g j f", j=3),
                            in0=MSG[:, :, 128:256][:, :, None, :].to_broadcast(
                                [128, G, 3, 128]),
                            in1=vslice.rearrange("p (g j) -> p g j", j=3
                                ).to_broadcast([128, G, 3, 128]),
                            op=ALU.mult)
                        # scatter: accA += S^T @ [ds|t1], accB += S^T @ t2
                        for i in range(G):
                            t_loc = gt0 + i
                            first = t_loc == 0
                            last = t_loc == T_b - 1
                            lhsT = S_blk[:, t_loc * 128:(t_loc + 1) * 128]
                            nc.tensor.matmul(
                                out=accA[:], lhsT=lhsT, rhs=MSG[:, i, 256:768],
                                start=first, stop=last, skip_group_check=True)
                            nc.tensor.matmul(
                                out=accB[:], lhsT=lhsT, rhs=MSG[:, i, 768:1152],
                                start=first, stop=last, skip_group_check=True)

                    # drain block: ds rows, dvec = accA[:,128:512] + accB
                    ds_sb = op_.tile([128, 128], dt.float32, tag="dssb")
                    dv_sb = op_.tile([128, 384], dt.float32, tag="dvsb")
                    b_sb = op_.tile([128, 384], dt.float32, tag="bsb")
                    bsl = slice(b * 128, (b + 1) * 128)
                    nc.scalar.activation(out=ds_sb[:], in_=accA[:, 0:128], func=ACTF.Copy)
                    nc.scalar.activation(out=b_sb[:], in_=accB[:], func=ACTF.Copy)
                    nc.vector.tensor_tensor(
                        out=dv_sb[:], in0=accA[:, 128:512], in1=b_sb[:], op=ALU.add)
                    nc.sync.dma_start(out=ds_out_d[bsl, :], in_=ds_sb[:])
                    nc.sync.dma_start(out=dv_out_d[bsl, :], in_=dv_sb[:])

                    t_base += T_b

    nc.compile()
    _PROGRAM_CACHE[key] = nc
    return nc


def kernel(s, vec, edge_vector, edge_distance, edge_rbf,
           Ws1, bs1, Ws2, bs2, Wrbf, brbf,
           edge_indexes, cutoff_dist):
    from concourse.bass_utils import run_bass_kernel_spmd

    s = np.asarray(s, np.float32)
    vec = np.asarray(vec, np.float32)
    edge_vector = np.asarray(edge_vector, np.float32)
    edge_distance = np.asarray(edge_distance, np.float32)
    edge_rbf = np.asarray(edge_rbf, np.float32)
    Ws1 = np.asarray(Ws1, np.float32)
    bs1 = np.asarray(bs1, np.float32)
    Ws2 = np.asarray(Ws2, np.float32)
    bs2 = np.asarray(bs2, np.float32)
    Wrbf = np.asarray(Wrbf, np.float32)
    brbf = np.asarray(brbf, np.float32)
    edge_indexes = np.asarray(edge_indexes)
    rc = float(np.asarray(cutoff_dist))

    E = edge_indexes.shape[1]
    n_nodes, nf = s.shape
    assert (n_nodes, nf) == (N_NODES, F)

    dst = edge_indexes[0].astype(np.int64)
    src = edge_indexes[1].astype(np.int64)

    # permutation of the 3F output features: [Wvv | Wvs | Ws]
    perm = np.concatenate([np.arange(F, 2 * F), np.arange(2 * F, 3 * F),
                           np.arange(0, F)])
    Ws2p = Ws2[perm]
    Wrbfp = Wrbf[perm]
    brbfp = brbf[perm]
    bs2p = bs2[perm]
    brbf_nz = bool(np.any(brbfp != 0))
    bs2_nz = bool(np.any(bs2p != 0))
    KRBF = 21 if brbf_nz else 20

    # ---- bucket edges by (core, block) ----
    core_of = dst // NPC
    blk_of = (dst % NPC) // BLKW
    # edge counts per (core, block)
    counts = np.zeros((N_CORES, NBLK), np.int64)
    np.add.at(counts, (core_of, blk_of), 1)
    T_bs = []
    for b in range(NBLK):
        need = int(np.ceil(counts[:, b].max() / 128.0))
        need = max(GCH, ((need + GCH - 1) // GCH) * GCH)
        T_bs.append(need)
    T_total = sum(T_bs)

    # order edges: bucket sort by (core, block)
    order = np.lexsort((blk_of, core_of))

    nc = _build_program(T_bs, brbf_nz, bs2_nz)

    # ---- shared (per-core identical) arrays ----
    s_pad = np.zeros((NT_PAD, F), np.float32)
    s_pad[:N_NODES] = s
    sT = s_pad.T.copy()                       # [128, NT_PAD]
    sT_hi, sT_lo = _split_bf16(sT)
    w1T_hi, w1T_lo = _split_bf16(Ws1.T.copy())      # lhsT[k][h] = Ws1[h][k]
    w2T_hi, w2T_lo = _split_bf16(Ws2p.T.copy())     # rhs[k][n] = Ws2p[n][k]
    bs1_col = bs1.reshape(F, 1).astype(np.float32)
    vec_tab = vec.reshape(N_NODES, 3 * F).astype(BF16)
    wrbf_rep = np.zeros((128, 384), BF16)
    wr = np.zeros((KRBF, 384), np.float32)
    wr[:RBF] = Wrbfp.T
    if brbf_nz:
        wr[RBF] = brbfp
    wrbf_rep[0:KRBF] = wr.astype(BF16)
    wrbf_rep[32:32 + KRBF] = wr.astype(BF16)
    rcinfo = np.zeros((128, 4), np.float32)
    rcinfo[:, 2] = np.pi / 2
    rcinfo[:, 0] = 1.0 / rc
    rcinfo[:, 1] = rc
    shared = {
        "sT_hi": sT_hi, "sT_lo": sT_lo,
        "w1T_hi": w1T_hi, "w1T_lo": w1T_lo,
        "w2T_hi": w2T_hi, "w2T_lo": w2T_lo,
        "bs1c": bs1_col, "vec_tab": vec_tab, "wrbf_rep": wrbf_rep,
        "rcinfo": rcinfo,
    }
    if bs2_nz:
        hi, lo = _split_bf16(bs2p.reshape(1, 384))
        shared["bs2hl"] = np.concatenate([hi, lo], 0)
        shared["ones2"] = np.ones((2, 128), BF16)

    # ---- per-core packed arrays ----
    in_maps = []
    starts = np.zeros((N_CORES, NBLK), np.int64)
    flat = counts.cumsum().reshape(N_CORES, NBLK)
    starts.reshape(-1)[1:] = flat.reshape(-1)[:-1]

    for c in range(N_CORES):
        n_e = T_total * 128
        e_src = np.zeros(n_e, np.int64)
        e_dstrel = np.zeros(n_e, np.int64)
        e_rbf = np.zeros((n_e, RBF), np.float32)
        e_dist = np.full(n_e, 2.0 * rc, np.float32)   # dummy: fcut=0
        e_ev = np.zeros((n_e, 3), np.float32)
        t0 = 0
        for b in range(NBLK):
            cnt = int(counts[c, b])
            sel = order[starts[c, b]:starts[c, b] + cnt]
            o = t0 * 128
            e_src[o:o + cnt] = src[sel]
            e_dstrel[o:o + cnt] = dst[sel] - (c * NPC + b * BLKW)
            e_rbf[o:o + cnt] = edge_rbf[sel]
            e_dist[o:o + cnt] = edge_distance[sel]
            e_ev[o:o + cnt] = edge_vector[sel]
            t0 += T_bs[b]

        # idx layout: per gather chunk of GCH tiles, per-block chunking
        idx_all = np.zeros((128, T_total * 8), np.int16)
        t0 = 0
        for b in range(NBLK):
            T_b = T_bs[b]
            for chi in range((T_b + GCH - 1) // GCH):
                ct0 = chi * GCH
                ct1 = min(ct0 + GCH, T_b)
                loc = e_src[(t0 + ct0) * 128:(t0 + ct1) * 128].astype(np.int16)
                w = _wrap_idx(loc, (ct1 - ct0) * 8)
                idx_all[:, (t0 + ct0) * 8:(t0 + ct1) * 8] = w
            t0 += T_bs[b]

        # rbf2T: tile t -> group g=t%2 partitions [32g:32g+KRBF], cols [q*128:(q+1)*128]
        ep = e_rbf.reshape(T_total, 128, RBF)
        rbf2T = np.zeros((64, (T_total // 2) * 128), BF16)
        raug = np.zeros((T_total, KRBF, 128), np.float32)
        raug[:, :RBF, :] = ep.transpose(0, 2, 1)
        if brbf_nz:
            raug[:, RBF, :] = 1.0
        raug_bf = raug.astype(BF16)
        for gpos in range(2):
            sel_t = np.arange(gpos, T_total, 2)
            blkv = raug_bf[sel_t]                       # [T/2, KRBF, 128]
            rbf2T[32 * gpos:32 * gpos + KRBF] = (
                blkv.transpose(1, 0, 2).reshape(KRBF, -1))

        # S one-hot [128 edges, 128 nodes] per tile
        dr = e_dstrel.reshape(T_total, 128)
        S_bf = np.zeros((128, T_total, 128), BF16)
        tt, pp = np.meshgrid(np.arange(T_total), np.arange(128), indexing="ij")
        S_bf[pp.reshape(-1), tt.reshape(-1), dr.reshape(-1)] = 1.0
        S_bf = S_bf.reshape(128, T_total * 128)

        dist_pt = e_dist.reshape(T_total, 128).T.copy()          # [128, T]
        ev_pt = np.ascontiguousarray(
            e_ev.reshape(T_total, 128, 3).transpose(1, 0, 2)).reshape(128, T_total * 3)

        m = dict(shared)
        m.update({
            "rbf2T": rbf2T, "idx_all": idx_all, "S_bf": S_bf,
            "dist_pt": dist_pt, "ev_pt": ev_pt,
        })
        in_maps.append(m)

    trace = bool(int(os.environ.get("BASS_KERNEL_TRACE", "0")))
    kw = {}
    if trace:
        _install_ntff_hook_shim()
        kw = dict(trace=True, trace_cores=[0], tmpdir=os.environ.get(
            "BASS_KERNEL_TRACE_DIR", "/tmp/gnn_trace"))
        os.makedirs(kw["tmpdir"], exist_ok=True)
    res = run_bass_kernel_spmd(nc, in_maps, core_ids=list(range(N_CORES)), **kw)
    if trace and res.exec_time_ns is not None:
        print(f"HW exec time: {res.exec_time_ns} ns")
        kernel.last_exec_ns = res.exec_time_ns

    ds = np.zeros((N_NODES, F), np.float32)
    dvec = np.zeros((N_NODES, 3, F), np.float32)
    for c in range(N_CORES):
        r = res.results[c]
        ds[c * NPC:(c + 1) * NPC] = r["ds_out"][:NPC]
        dvec[c * NPC:(c + 1) * NPC] = r["dv_out"][:NPC].reshape(NPC, 3, F)
    return ds, dvec


# revision 14
# speedup vs baseline: 2.1697x; 2.1697x over previous
"""Trainium2 Bass kernel for the MessageBlock GNN message-passing layer.

Strategy (8 NeuronCores, no collectives):
  - Sort edges by destination node on host; shard by dst range: core c owns
    nodes [c*1250, (c+1)*1250), split into 10 blocks of 128 nodes.
  - Node MLP phi = Linear(SiLU(Linear(s))) computed per *node* (not per edge)
    on every core (redundant but cheap), written to a DRAM table in bf16.
  - Per edge tile (128 edges): dma_gather phi[src], vec[src] (bf16 rows),
    rbf @ WrbfT via row-packed matmuls, fcut fold via ScalarE, elementwise
    message assembly on VectorE/GpSimd, then scatter-add via one-hot S
    matmuls accumulating in PSUM per 128-node block.
  - Each core writes its own disjoint slice of ds/dvec; host concatenates.
"""

import sys
import os

for _p in ("/opt/trn_rl_repo", "/root/.axon_site/_ro/trn_rl_repo"):
    if os.path.isdir(_p) and _p not in sys.path:
        sys.path.insert(0, _p)

import numpy as np
import ml_dtypes

BF16 = ml_dtypes.bfloat16

N_NODES = 10000
F = 128
RBF = 20
N_CORES = 8
NPC = 1250            # nodes per core
NBLK = 10             # node blocks per core (128 nodes each; last=98)
BLKW = 128            # block width (nodes)
G = 4                 # tiles per elementwise supergroup
GCH = 8               # tiles per gather chunk
NT_PAD = 10240        # padded node count for the MLP phase (80 tiles of 128)


def _split_bf16(x):
    hi = x.astype(BF16)
    lo = (x.astype(np.float32) - hi.astype(np.float32)).astype(BF16)
    return hi, lo


def _wrap_idx(idx_i16, ncols):
    """dma_gather index layout: idx i -> partition i%16, col i//16,
    replicated across the 8 Q7 cores (partitions 16..127)."""
    out = np.zeros((128, ncols), np.int16)
    n = len(idx_i16)
    cols = (n + 15) // 16
    tmp = np.full(16 * cols, 0, np.int16)
    tmp[:n] = idx_i16
    out[:16, :cols] = tmp.reshape(cols, 16).T
    return np.tile(out[:16], (8, 1))



def _install_ntff_hook_shim():
    """The image's antenv package lacks axon_hooks; synthesize it and
    register the boot-provided ctypes NTFF profiling hook."""
    import types
    import antenv
    if "antenv.axon_hooks" in sys.modules:
        return
    mod = types.ModuleType("antenv.axon_hooks")
    mod._hook = None

    def set_axon_ntff_profile_hook(h):
        mod._hook = h

    def get_axon_ntff_profile_hook():
        return mod._hook

    mod.set_axon_ntff_profile_hook = set_axon_ntff_profile_hook
    mod.get_axon_ntff_profile_hook = get_axon_ntff_profile_hook
    sys.modules["antenv.axon_hooks"] = mod
    antenv.axon_hooks = mod
    try:
        from trn_agent_boot.trn_boot import _ntff_profile_via_ctypes
        hook = _ntff_profile_via_ctypes("/opt/axon/libaxon_pjrt.so")
        if hook is not None:
            mod._hook = hook
    except Exception as e:
        print(f"ntff hook shim failed: {e}")


_PROGRAM_CACHE = {}


def _build_program(T_bs, brbf_nz, bs2_nz):
    key = (tuple(T_bs), brbf_nz, bs2_nz, os.environ.get("GNN_PREP", "1"))
    if key in _PROGRAM_CACHE:
        return _PROGRAM_CACHE[key]

    import concourse.bass as bass
    import concourse.bacc as bacc
    import concourse.mybir as mybir
    import concourse.tile as tile
    from concourse.tile import add_dep_helper

    dt = mybir.dt
    ALU = mybir.AluOpType
    ACTF = mybir.ActivationFunctionType

    T_total = sum(T_bs)
    KRBF = 21 if brbf_nz else 20

    nc = bacc.Bacc("TRN2", target_bir_lowering=False, debug=False, num_swdge_queues=4)

    # ---- external inputs (identical shapes on every core) ----
    sT_hi_d = nc.dram_tensor("sT_hi", [128, NT_PAD], dt.bfloat16, kind="ExternalInput")
    sT_lo_d = nc.dram_tensor("sT_lo", [128, NT_PAD], dt.bfloat16, kind="ExternalInput")
    w1T_hi_d = nc.dram_tensor("w1T_hi", [128, 128], dt.bfloat16, kind="ExternalInput")
    w1T_lo_d = nc.dram_tensor("w1T_lo", [128, 128], dt.bfloat16, kind="ExternalInput")
    w2T_hi_d = nc.dram_tensor("w2T_hi", [128, 384], dt.bfloat16, kind="ExternalInput")
    w2T_lo_d = nc.dram_tensor("w2T_lo", [128, 384], dt.bfloat16, kind="ExternalInput")
    bs1_d = nc.dram_tensor("bs1c", [128, 1], dt.float32, kind="ExternalInput")
    if bs2_nz:
        bs2hl_d = nc.dram_tensor("bs2hl", [2, 384], dt.bfloat16, kind="ExternalInput")
        ones2_d = nc.dram_tensor("ones2", [2, 128], dt.bfloat16, kind="ExternalInput")
    vec_tab_d = nc.dram_tensor("vec_tab", [N_NODES, 384], dt.bfloat16, kind="ExternalInput")
    wrbf_d = nc.dram_tensor("wrbf_rep", [128, 384], dt.bfloat16, kind="ExternalInput")
    rbf2T_d = nc.dram_tensor("rbf2T", [64, (T_total // 2) * 128], dt.bfloat16, kind="ExternalInput")
    idx_d = nc.dram_tensor("idx_all", [128, T_total * 8], dt.int16, kind="ExternalInput")
    S_d = nc.dram_tensor("S_bf", [128, T_total * 128], dt.bfloat16, kind="ExternalInput")
    dist_d = nc.dram_tensor("dist_pt", [128, T_total], dt.float32, kind="ExternalInput")
    ev_d = nc.dram_tensor("ev_pt", [128, T_total * 3], dt.float32, kind="ExternalInput")
    rcinfo_d = nc.dram_tensor("rcinfo", [128, 4], dt.float32, kind="ExternalInput")

    ds_out_d = nc.dram_tensor("ds_out", [NBLK * 128, 128], dt.float32, kind="ExternalOutput")
    dv_out_d = nc.dram_tensor("dv_out", [NBLK * 128, 384], dt.float32, kind="ExternalOutput")

    # internal DRAM: combined gather table rows = [phi(384) | vec(384)] bf16
    tab_d = nc.dram_tensor("tab", [NT_PAD, 768], dt.bfloat16)

    NCHUNK = NT_PAD // 512    # 20 node chunks for mm1
    NTILES = NT_PAD // 128    # 80 node tiles for mm2

    with tile.TileContext(nc) as tc:
        # ---------------- constants / persistent tiles ----------------
        with tc.tile_pool(name="const", bufs=1) as cp:
            w1T_hi = cp.tile([128, 128], dt.bfloat16, tag="w1hi")
            w1T_lo = cp.tile([128, 128], dt.bfloat16, tag="w1lo")
            w2T_hi = cp.tile([128, 384], dt.bfloat16, tag="w2hi")
            w2T_lo = cp.tile([128, 384], dt.bfloat16, tag="w2lo")
            bs1 = cp.tile([128, 1], dt.float32, tag="bs1")
            wrbf = cp.tile([128, 384], dt.bfloat16, tag="wrbf")
            rbf2T = cp.tile([64, (T_total // 2) * 128], dt.bfloat16, tag="rbf2T")
            idx_all = cp.tile([128, T_total * 8], dt.int16, tag="idx")
            dist = cp.tile([128, T_total], dt.float32, tag="dist")
            ev = cp.tile([128, T_total * 3], dt.float32, tag="ev")
            rcinfo = cp.tile([128, 4], dt.float32, tag="rcinfo")
            fcut = cp.tile([128, T_total], dt.float32, tag="fcut")
            vecn = cp.tile([128, T_total * 3], dt.float32, tag="vecn")
            scr0 = cp.tile([128, T_total], dt.float32, tag="scr0")
            scr1 = cp.tile([128, T_total], dt.float32, tag="scr1")
            scr2 = cp.tile([128, T_total], dt.float32, tag="scr2")
            if bs2_nz:
                bs2hl = cp.tile([2, 384], dt.bfloat16, tag="bs2hl")
                ones2 = cp.tile([2, 128], dt.bfloat16, tag="ones2")

            nc.sync.dma_start(out=w1T_hi[:], in_=w1T_hi_d[:])
            nc.sync.dma_start(out=w1T_lo[:], in_=w1T_lo_d[:])
            nc.sync.dma_start(out=w2T_hi[:], in_=w2T_hi_d[:])
            nc.sync.dma_start(out=w2T_lo[:], in_=w2T_lo_d[:])
            nc.sync.dma_start(out=bs1[:], in_=bs1_d[:])
            nc.sync.dma_start(out=wrbf[:], in_=wrbf_d[:])
            nc.sync.dma_start(out=rbf2T[:], in_=rbf2T_d[:])
            nc.scalar.dma_start(out=idx_all[:], in_=idx_d[:])
            nc.scalar.dma_start(out=dist[:], in_=dist_d[:])
            nc.scalar.dma_start(out=ev[:], in_=ev_d[:])
            nc.sync.dma_start(out=rcinfo[:], in_=rcinfo_d[:])
            # vec half of the gather table (phi half written by node phase)
            nc.gpsimd.dma_start(out=tab_d[0:N_NODES, 384:768], in_=vec_tab_d[:])
            if bs2_nz:
                nc.sync.dma_start(out=bs2hl[:], in_=bs2hl_d[:])
                nc.sync.dma_start(out=ones2[:], in_=ones2_d[:])

            # ---------------- edge-scalar precompute ----------------
            # fcut = 0.5*(cos(pi*d/rc)+1)*(d<rc) = (1 - sin^2(pi*d/(2rc)))*(d<rc)
            # sin((pi/2)u) = u * Q(u^2), Taylor with 7 terms (|err| ~ 1e-9 on [0,1]).
            # rcinfo col0 = 1/rc, col1 = rc.  ACT Sin LUT is inaccurate; use DVE poly.
            import math
            acoef = [(-1.0) ** kk * (math.pi / 2) ** (2 * kk + 1) / math.factorial(2 * kk + 1)
                     for kk in range(7)]
            u_t = scr0
            u2_t = scr1
            q_t = scr2
            nc.vector.tensor_scalar(
                out=u_t[:], in0=dist[:], scalar1=rcinfo[:, 0:1], scalar2=None,
                op0=ALU.mult)
            nc.vector.tensor_tensor(out=u2_t[:], in0=u_t[:], in1=u_t[:], op=ALU.mult)
            nc.vector.tensor_scalar(
                out=q_t[:], in0=u2_t[:], scalar1=float(acoef[6]), scalar2=float(acoef[5]),
                op0=ALU.mult, op1=ALU.add)
            for kk in (4, 3, 2, 1, 0):
                nc.vector.tensor_tensor(out=q_t[:], in0=q_t[:], in1=u2_t[:], op=ALU.mult)
                nc.vector.tensor_scalar(
                    out=q_t[:], in0=q_t[:], scalar1=float(acoef[kk]), scalar2=None,
                    op0=ALU.add)
            nc.vector.tensor_tensor(out=q_t[:], in0=q_t[:], in1=u_t[:], op=ALU.mult)
            nc.vector.tensor_tensor(out=q_t[:], in0=q_t[:], in1=q_t[:], op=ALU.mult)
            nc.vector.tensor_scalar(
                out=q_t[:], in0=q_t[:], scalar1=-1.0, scalar2=1.0,
                op0=ALU.mult, op1=ALU.add)
            nc.vector.tensor_scalar(
                out=scr0[:], in0=dist[:], scalar1=rcinfo[:, 1:2], scalar2=None,
                op0=ALU.is_lt)
            nc.vector.tensor_tensor(out=fcut[:], in0=q_t[:], in1=scr0[:], op=ALU.mult)
            # vecn[:, 3t+j] = ev[:, 3t+j] / dist[:, t]
            nc.vector.reciprocal(out=scr1[:], in_=dist[:])
            ev3 = ev[:].rearrange("p (t j) -> p t j", j=3)
            vecn3 = vecn[:].rearrange("p (t j) -> p t j", j=3)
            for j in range(3):
                nc.vector.tensor_tensor(
                    out=vecn3[:, :, j], in0=ev3[:, :, j], in1=scr1[:], op=ALU.mult
                )

            # ---------------- phase 1: node MLP ----------------
            with tc.tile_pool(name="nodephase", bufs=1) as npp, \
                 tc.tile_pool(name="phistage", bufs=4) as php, \
                 tc.tile_pool(name="ps_node", bufs=2, space="PSUM") as psn:
                sT_hi = npp.tile([128, NT_PAD], dt.bfloat16, tag="sThi")
                sT_lo = npp.tile([128, NT_PAD], dt.bfloat16, tag="sTlo")
                h_bf = npp.tile([128, NT_PAD], dt.bfloat16, tag="hbf")
                nc.sync.dma_start(out=sT_hi[:], in_=sT_hi_d[:])
                nc.sync.dma_start(out=sT_lo[:], in_=sT_lo_d[:])

                for ch in range(NCHUNK):
                    sl = slice(ch * 512, (ch + 1) * 512)
                    h_ps = psn.tile([128, 512], dt.float32, tag="h")
                    nc.tensor.matmul(out=h_ps[:], lhsT=w1T_hi[:], rhs=sT_hi[:, sl],
                                     start=True, stop=False)
                    nc.tensor.matmul(out=h_ps[:], lhsT=w1T_hi[:], rhs=sT_lo[:, sl],
                                     start=False, stop=False)
                    nc.tensor.matmul(out=h_ps[:], lhsT=w1T_lo[:], rhs=sT_hi[:, sl],
                                     start=False, stop=True)
                    nc.scalar.activation(out=h_bf[:, sl], in_=h_ps[:],
                                         func=ACTF.Silu, bias=bs1[:, 0:1], scale=1.0)

                for ntile in range(NTILES):
                    nsl = slice(ntile * 128, (ntile + 1) * 128)
                    phi_ps = psn.tile([128, 384], dt.float32, tag="phi")
                    nc.tensor.matmul(out=phi_ps[:], lhsT=h_bf[:, nsl], rhs=w2T_hi[:],
                                     start=True, stop=False)
                    nc.tensor.matmul(out=phi_ps[:], lhsT=h_bf[:, nsl], rhs=w2T_lo[:],
                                     start=False, stop=not bs2_nz)
                    if bs2_nz:
                        nc.tensor.matmul(out=phi_ps[:], lhsT=ones2[:], rhs=bs2hl[:],
                                         start=False, stop=True)
                    phi_sb = php.tile([128, 384], dt.bfloat16, tag="phisb")
                    if ntile % 2 == 0:
                        nc.scalar.activation(out=phi_sb[:], in_=phi_ps[:], func=ACTF.Copy)
                    else:
                        nc.vector.tensor_copy(out=phi_sb[:], in_=phi_ps[:])
                    nc.sync.dma_start(out=tab_d[nsl, 0:384], in_=phi_sb[:])

            # fence: a tiny read of tab_d orders gather triggers after all
            # table writers (phi-phase DMAs + vec copy); the PREPARE_ONLY
    # trigger path does not carry the DRAM RAW itself.
            fence_t = cp.tile([128, 8], dt.bfloat16, tag="fence")
            fence_ins = nc.sync.dma_start(out=fence_t[:], in_=tab_d[0:128, 0:8]).ins

            # ---------------- phase 2: edge processing ----------------
            with tc.tile_pool(name="gath", bufs=6) as gp, \
                 tc.tile_pool(name="sblk", bufs=2) as sp, \
                 tc.tile_pool(name="msgp", bufs=2) as mp, \
                 tc.tile_pool(name="wbfp", bufs=2) as wp, \
                 tc.tile_pool(name="outp", bufs=2) as op_, \
                 tc.tile_pool(name="ps_w", bufs=2, space="PSUM") as psw, \
                 tc.tile_pool(name="ps_acc", bufs=2, space="PSUM") as psa:

                t_base = 0
                gq = 0
                GBUFS = 6
                use_prep = bool(int(os.environ.get("GNN_PREP", "1")))
                slot_readers = [[] for _ in range(GBUFS)]
                for b in range(NBLK):
                    T_b = T_bs[b]
                    nchunks = T_b // GCH
                    chunk_tiles = []
                    chunk_waits = []
                    chunk_slots = []
                    for chi in range(nchunks):
                        ct0 = chi * GCH
                        nidx = GCH * 128
                        isl = slice((t_base + ct0) * 8, (t_base + ct0 + GCH) * 8)
                        gt = gp.tile([128, GCH, 768], dt.bfloat16, tag="gath")
                        slot = gq % GBUFS
                        if use_prep:
                            gsem = nc.alloc_semaphore(f"gsem_{b}_{chi}")
                            nc.gpsimd.dma_gather(
                                gt[:], tab_d[:], idx_all[:, isl],
                                nidx, nidx, 768, single_packet=False,
                                queue_num=gq % 4, prepare_only=True, sem=gsem)
                            trig = nc.gpsimd.trigger_dma(count=1, queue_num=gq % 4)
                            add_dep_helper(trig.ins, fence_ins, sync=True,
                                           reason="gather after table writes")
                            for r in slot_readers[slot]:
                                add_dep_helper(trig.ins, r, sync=True,
                                               reason="WAR prior chunk readers")
                            w = nc.vector.wait_ge(gsem, 16)
                            chunk_waits.append(w.ins)
                        else:
                            nc.gpsimd.dma_gather(
                                gt[:], tab_d[:], idx_all[:, isl],
                                nidx, nidx, 768, single_packet=False,
                                queue_num=gq % 4)
                            chunk_waits.append(None)
                        slot_readers[slot] = []
                        chunk_slots.append(slot)
                        gq += 1
                        chunk_tiles.append(gt)

                    S_blk = sp.tile([128, T_b * 128], dt.bfloat16, tag="S")
                    nc.scalar.dma_start(
                        out=S_blk[:], in_=S_d[:, t_base * 128:(t_base + T_b) * 128])

                    accA = psa.tile([128, 512], dt.float32, tag="accA")
                    accB = psa.tile([128, 384], dt.float32, tag="accB")

                    for g in range(T_b // G):
                        gt0 = g * G            # tile index within block
                        MSG = mp.tile([128, G, 1152], dt.bfloat16, tag="MSG")
                        W_bf = wp.tile([128, G, 384], dt.bfloat16, tag="Wbf")
                        wps = psw.tile([128, 2, 512], dt.float32, tag="wps")
                        # rbf matmuls: pack 2 tiles into PE rows (0,32)
                        for i in range(G):
                            t_glob = t_base + gt0 + i
                            grp = t_glob % 2
                            q = t_glob // 2
                            nc.tensor.matmul(
                                out=wps[:, i % 2, 0:384],
                                lhsT=rbf2T[32 * grp:32 * grp + KRBF,
                                           q * 128:(q + 1) * 128],
                                rhs=wrbf[32 * grp:32 * grp + KRBF, :],
                                start=True, stop=True)
                            if i % 2 == 1 or i == G - 1:
                                for ii in (i - i % 2, i) if i % 2 == 1 else (i,):
                                    tt = t_base + gt0 + ii
                                    nc.scalar.activation(
                                        out=W_bf[:, ii, :], in_=wps[:, ii % 2, 0:384],
                                        func=ACTF.Copy, scale=fcut[:, tt:tt + 1])
                                if i != G - 1:
                                    wps = psw.tile([128, 2, 512], dt.float32, tag="wps")
                        chi = gt0 // GCH
                        gt = chunk_tiles[chi]
                        goff = gt0 % GCH
                        readers = slot_readers[chunk_slots[chi]]
                        # phiW = phi * W  -> MSG[:, :, 0:384]  ([Wvv|Wvs|ds])
                        phiw_tt = nc.vector.tensor_tensor(
                            out=MSG[:, :, 0:384],
                            in0=gt[:, goff:goff + G, 0:384],
                            in1=W_bf[:], op=ALU.mult)
                        if chunk_waits[chi] is not None:
                            add_dep_helper(phiw_tt.ins, chunk_waits[chi], sync=False,
                                           reason="gather data wait")
                        readers.append(phiw_tt.ins)
                        # t1_j = vec_j * Wvv -> MSG[:, :, 384+128j : ...]
                        for j in range(3):
                            eng = nc.vector
                            t1_tt = eng.tensor_tensor(
                                out=MSG[:, :, 384 + 128 * j:384 + 128 * (j + 1)],
                                in0=gt[:, goff:goff + G,
                                       384 + 128 * j:384 + 128 * (j + 1)],
                                in1=MSG[:, :, 0:128], op=ALU.mult)
                            readers.append(t1_tt.ins)
                        # t2 = vecn_bcast * Wvs_bcast in one TT over [128, G, 3, 128]
                        t_glob0 = t_base + gt0
                        vslice = vecn[:, 3 * t_glob0:3 * (t_glob0 + G)]
                        nc.vector.tensor_tensor(
                            out=MSG[:, :, 768:1152].rearrange("p g (j f) -> p g j f", j=3),
                            in0=MSG[:, :, 128:256][:, :, None, :].to_broadcast(
                                [128, G, 3, 128]),
                            in1=vslice.rearrange("p (g j) -> p g j", j=3
                                ).to_broadcast([128, G, 3, 128]),
                            op=ALU.mult)
                        # scatter: accA += S^T @ [ds|t1], accB += S^T @ t2
                        for i in range(G):
                            t_loc = gt0 + i
                            first = t_loc == 0
                            last = t_loc == T_b - 1
                            lhsT = S_blk[:, t_loc * 128:(t_loc + 1) * 128]
                            nc.tensor.matmul(
                                out=accA[:], lhsT=lhsT, rhs=MSG[:, i, 256:768],
                                start=first, stop=last, skip_group_check=True)
                            nc.tensor.matmul(
                                out=accB[:], lhsT=lhsT, rhs=MSG[:, i, 768:1152],
                                start=first, stop=last, skip_group_check=True)

                    # drain block: ds rows, dvec = accA[:,128:512] + accB
                    ds_sb = op_.tile([128, 128], dt.float32, tag="dssb")
                    dv_sb = op_.tile([128, 384], dt.float32, tag="dvsb")
                    b_sb = op_.tile([128, 384], dt.float32, tag="bsb")
                    bsl = slice(b * 128, (b + 1) * 128)
                    nc.scalar.activation(out=ds_sb[:], in_=accA[:, 0:128], func=ACTF.Copy)
                    nc.scalar.activation(out=b_sb[:], in_=accB[:], func=ACTF.Copy)
                    nc.vector.tensor_tensor(
                        out=dv_sb[:], in0=accA[:, 128:512], in1=b_sb[:], op=ALU.add)
                    nc.sync.dma_start(out=ds_out_d[bsl, :], in_=ds_sb[:])
                    nc.sync.dma_start(out=dv_out_d[bsl, :], in_=dv_sb[:])

                    t_base += T_b

    nc.compile()
    _PROGRAM_CACHE[key] = nc
    return nc


def kernel(s, vec, edge_vector, edge_distance, edge_rbf,
           Ws1, bs1, Ws2, bs2, Wrbf, brbf,
           edge_indexes, cutoff_dist):
    from concourse.bass_utils import run_bass_kernel_spmd

    s = np.asarray(s, np.float32)
    vec = np.asarray(vec, np.float32)
    edge_vector = np.asarray(edge_vector, np.float32)
    edge_distance = np.asarray(edge_distance, np.float32)
    edge_rbf = np.asarray(edge_rbf, np.float32)
    Ws1 = np.asarray(Ws1, np.float32)
    bs1 = np.asarray(bs1, np.float32)
    Ws2 = np.asarray(Ws2, np.float32)
    bs2 = np.asarray(bs2, np.float32)
    Wrbf = np.asarray(Wrbf, np.float32)
    brbf = np.asarray(brbf, np.float32)
    edge_indexes = np.asarray(edge_indexes)
    rc = float(np.asarray(cutoff_dist))

    E = edge_indexes.shape[1]
    n_nodes, nf = s.shape
    assert (n_nodes, nf) == (N_NODES, F)

    dst = edge_indexes[0].astype(np.int64)
    src = edge_indexes[1].astype(np.int64)

    # permutation of the 3F output features: [Wvv | Wvs | Ws]
    perm = np.concatenate([np.arange(F, 2 * F), np.arange(2 * F, 3 * F),
                           np.arange(0, F)])
    Ws2p = Ws2[perm]
    Wrbfp = Wrbf[perm]
    brbfp = brbf[perm]
    bs2p = bs2[perm]
    brbf_nz = bool(np.any(brbfp != 0))
    bs2_nz = bool(np.any(bs2p != 0))
    KRBF = 21 if brbf_nz else 20

    # ---- bucket edges by (core, block) ----
    core_of = dst // NPC
    blk_of = (dst % NPC) // BLKW
    # edge counts per (core, block)
    counts = np.zeros((N_CORES, NBLK), np.int64)
    np.add.at(counts, (core_of, blk_of), 1)
    T_bs = []
    for b in range(NBLK):
        need = int(np.ceil(counts[:, b].max() / 128.0))
        need = max(GCH, ((need + GCH - 1) // GCH) * GCH)
        T_bs.append(need)
    T_total = sum(T_bs)

    # order edges: bucket sort by (core, block)
    order = np.lexsort((blk_of, core_of))

    nc = _build_program(T_bs, brbf_nz, bs2_nz)

    # ---- shared (per-core identical) arrays ----
    s_pad = np.zeros((NT_PAD, F), np.float32)
    s_pad[:N_NODES] = s
    sT = s_pad.T.copy()                       # [128, NT_PAD]
    sT_hi, sT_lo = _split_bf16(sT)
    w1T_hi, w1T_lo = _split_bf16(Ws1.T.copy())      # lhsT[k][h] = Ws1[h][k]
    w2T_hi, w2T_lo = _split_bf16(Ws2p.T.copy())     # rhs[k][n] = Ws2p[n][k]
    bs1_col = bs1.reshape(F, 1).astype(np.float32)
    vec_tab = vec.reshape(N_NODES, 3 * F).astype(BF16)
    wrbf_rep = np.zeros((128, 384), BF16)
    wr = np.zeros((KRBF, 384), np.float32)
    wr[:RBF] = Wrbfp.T
    if brbf_nz:
        wr[RBF] = brbfp
    wrbf_rep[0:KRBF] = wr.astype(BF16)
    wrbf_rep[32:32 + KRBF] = wr.astype(BF16)
    rcinfo = np.zeros((128, 4), np.float32)
    rcinfo[:, 2] = np.pi / 2
    rcinfo[:, 0] = 1.0 / rc
    rcinfo[:, 1] = rc
    shared = {
        "sT_hi": sT_hi, "sT_lo": sT_lo,
        "w1T_hi": w1T_hi, "w1T_lo": w1T_lo,
        "w2T_hi": w2T_hi, "w2T_lo": w2T_lo,
        "bs1c": bs1_col, "vec_tab": vec_tab, "wrbf_rep": wrbf_rep,
        "rcinfo": rcinfo,
    }
    if bs2_nz:
        hi, lo = _split_bf16(bs2p.reshape(1, 384))
        shared["bs2hl"] = np.concatenate([hi, lo], 0)
        shared["ones2"] = np.ones((2, 128), BF16)

    # ---- per-core packed arrays ----
    in_maps = []
    starts = np.zeros((N_CORES, NBLK), np.int64)
    flat = counts.cumsum().reshape(N_CORES, NBLK)
    starts.reshape(-1)[1:] = flat.reshape(-1)[:-1]

    for c in range(N_CORES):
        n_e = T_total * 128
        e_src = np.zeros(n_e, np.int64)
        e_dstrel = np.zeros(n_e, np.int64)
        e_rbf = np.zeros((n_e, RBF), np.float32)
        e_dist = np.full(n_e, 2.0 * rc, np.float32)   # dummy: fcut=0
        e_ev = np.zeros((n_e, 3), np.float32)
        t0 = 0
        for b in range(NBLK):
            cnt = int(counts[c, b])
            sel = order[starts[c, b]:starts[c, b] + cnt]
            o = t0 * 128
            e_src[o:o + cnt] = src[sel]
            e_dstrel[o:o + cnt] = dst[sel] - (c * NPC + b * BLKW)
            e_rbf[o:o + cnt] = edge_rbf[sel]
            e_dist[o:o + cnt] = edge_distance[sel]
            e_ev[o:o + cnt] = edge_vector[sel]
            t0 += T_bs[b]

        # idx layout: per gather chunk of GCH tiles, per-block chunking
        idx_all = np.zeros((128, T_total * 8), np.int16)
        t0 = 0
        for b in range(NBLK):
            T_b = T_bs[b]
            for chi in range((T_b + GCH - 1) // GCH):
                ct0 = chi * GCH
                ct1 = min(ct0 + GCH, T_b)
                loc = e_src[(t0 + ct0) * 128:(t0 + ct1) * 128].astype(np.int16)
                w = _wrap_idx(loc, (ct1 - ct0) * 8)
                idx_all[:, (t0 + ct0) * 8:(t0 + ct1) * 8] = w
            t0 += T_bs[b]

        # rbf2T: tile t -> group g=t%2 partitions [32g:32g+KRBF], cols [q*128:(q+1)*128]
        ep = e_rbf.reshape(T_total, 128, RBF)
        rbf2T = np.zeros((64, (T_total // 2) * 128), BF16)
        raug = np.zeros((T_total, KRBF, 128), np.float32)
        raug[:, :RBF, :] = ep.transpose(0, 2, 1)
        if brbf_nz:
            raug[:, RBF, :] = 1.0
        raug_bf = raug.astype(BF16)
        for gpos in range(2):
            sel_t = np.arange(gpos, T_total, 2)
            blkv = raug_bf[sel_t]                       # [T/2, KRBF, 128]
            rbf2T[32 * gpos:32 * gpos + KRBF] = (
                blkv.transpose(1, 0, 2).reshape(KRBF, -1))

        # S one-hot [128 edges, 128 nodes] per tile
        dr = e_dstrel.reshape(T_total, 128)
        S_bf = np.zeros((128, T_total, 128), BF16)
        tt, pp = np.meshgrid(np.arange(T_total), np.arange(128), indexing="ij")
        S_bf[pp.reshape(-1), tt.reshape(-1), dr.reshape(-1)] = 1.0
        S_bf = S_bf.reshape(128, T_total * 128)

        dist_pt = e_dist.reshape(T_total, 128).T.copy()          # [128, T]
        ev_pt = np.ascontiguousarray(
            e_ev.reshape(T_total, 128, 3).transpose(1, 0, 2)).reshape(128, T_total * 3)

        m = dict(shared)
        m.update({
            "rbf2T": rbf2T, "idx_all": idx_all, "S_bf": S_bf,
            "dist_pt": dist_pt, "ev_pt": ev_pt,
        })
        in_maps.append(m)

    trace = bool(int(os.environ.get("BASS_KERNEL_TRACE", "0")))
    kw = {}
    if trace:
        _install_ntff_hook_shim()
        kw = dict(trace=True, trace_cores=[0], tmpdir=os.environ.get(
            "BASS_KERNEL_TRACE_DIR", "/tmp/gnn_trace"))
        os.makedirs(kw["tmpdir"], exist_ok=True)
    res = run_bass_kernel_spmd(nc, in_maps, core_ids=list(range(N_CORES)), **kw)
    if trace and res.exec_time_ns is not None:
        print(f"HW exec time: {res.exec_time_ns} ns")
        kernel.last_exec_ns = res.exec_time_ns

    ds = np.zeros((N_NODES, F), np.float32)
    dvec = np.zeros((N_NODES, 3, F), np.float32)
    for c in range(N_CORES):
        r = res.results[c]
        ds[c * NPC:(c + 1) * NPC] = r["ds_out"][:NPC]
        dvec[c * NPC:(c + 1) * NPC] = r["dv_out"][:NPC].reshape(NPC, 3, F)
    return ds, dvec
